# Initial kernel scaffold
#
"""Bidirectional GRU (T=2048, B=32, F=H=512) on 8 Trainium2 NeuronCores.

Sharding: 8 cores = 2 directions x 4 batch quarters (B_local=8 per core).
Each core runs the full serial T-step recurrence for its slice; the reverse
direction cores receive time-flipped inputs so the same SPMD program serves
both directions.

Per-core layout: gates live as [128-gate-chunk partitions, batch] tiles so
all elementwise work runs at 128-partition occupancy. The input projection
gi = x @ w_ih^T + b is produced on-core in 64-step windows, double-buffered,
overlapped with the recurrence. Recurrent matmul: out[g,b] += w_hhT[k,g]^T
h[k,b] -> 48 small PE matmuls per step (weights stationary).
"""

import sys

import numpy as np

sys.path.insert(0, "/opt/trn_rl_repo")

T, B, F, H = 2048, 32, 512, 512
G = 3 * H
P = 128
BL = 8          # per-core batch
KO = F // P     # 4 contraction chunks
GM = G // P     # 12 gate chunks
W = 64          # steps per window (2 windows per loop body)

_BUILT = {}


def _build(T_, loop_mode, gi_f32r=False):
    import concourse.mybir as mybir
    import concourse.tile as tile
    from concourse import bacc
    from concourse.bass import ds

    f32 = mybir.dt.float32
    gidt = mybir.dt.float32r if gi_f32r else mybir.dt.float32
    AFT = mybir.ActivationFunctionType
    NW = T_ // W
    assert NW % 2 == 0
    WC = W * BL  # columns per window (512)

    nc = bacc.Bacc("TRN2", target_bir_lowering=False, debug=False)

    xT = nc.dram_tensor("xT", [F, T_ * BL + WC], gidt, kind="ExternalInput")
    wihT = nc.dram_tensor("wihT", [F, G], gidt, kind="ExternalInput")
    whhT = nc.dram_tensor("whhT", [F, G], f32, kind="ExternalInput")
    bihe = nc.dram_tensor("bihe", [P, GM], f32, kind="ExternalInput")
    bhhn = nc.dram_tensor("bhhn", [P, 4 * BL], f32, kind="ExternalInput")
    h0t = nc.dram_tensor("h0t", [P, KO * BL], f32, kind="ExternalInput")
    yT = nc.dram_tensor("yT", [H, T_ * BL], f32, kind="ExternalOutput")
    hTo = nc.dram_tensor("hTo", [P, KO * BL], f32, kind="ExternalOutput")

    xt_r = xT[:].rearrange("(ko p) n -> p ko n", p=P)
    yt_r = yT[:].rearrange("(ko p) n -> p ko n", p=P)

    with tile.TileContext(nc) as tc:
        with (
            tc.tile_pool(name="const", bufs=1) as const,
            tc.tile_pool(name="dbuf", bufs=1) as dbuf,
            tc.tile_pool(name="step", bufs=3) as steppool,
            tc.tile_pool(name="ghps", bufs=2, space="PSUM") as ghpool,
            tc.tile_pool(name="gips", bufs=2, space="PSUM") as gipool,
        ):
            wih_sb = const.tile([P, KO, G], gidt, tag="wih")
            whh_sb = const.tile([P, KO, G], f32, tag="whh")
            bihe_sb = const.tile([P, GM], f32, tag="bihe")
            bhhn_sb = const.tile([P, 4, BL], f32, tag="bhhn")
            h0_sb = const.tile([P, KO, BL], f32, tag="h0")
            hout_sb = const.tile([P, KO, BL], f32, tag="hout")

            xt_b = [dbuf.tile([P, KO, WC], gidt, tag=f"xt{i}") for i in range(2)]
            gi_b = [dbuf.tile([P, GM, WC], f32, tag=f"gi{i}") for i in range(2)]
            yw_b = [dbuf.tile([P, KO, WC], f32, tag=f"yw{i}") for i in range(2)]

            nc.sync.dma_start(wih_sb[:], wihT[:].rearrange("(ko p) g -> p ko g", p=P))
            nc.sync.dma_start(whh_sb[:], whhT[:].rearrange("(ko p) g -> p ko g", p=P))
            nc.sync.dma_start(bihe_sb[:], bihe[:])
            nc.sync.dma_start(bhhn_sb[:], bhhn[:].rearrange("p (k b) -> p k b", b=BL))
            nc.sync.dma_start(h0_sb[:], h0t[:].rearrange("p (k b) -> p k b", b=BL))

            def emit_gi(xtile, gitile):
                # gi^T window: [12 gate chunks x 128p, 512 cols] = wihT^T @ xT
                for m in range(GM):
                    ps = gipool.tile([P, WC], f32, tag="gips")
                    for k in range(KO):
                        nc.tensor.matmul(
                            ps[:],
                            lhsT=wih_sb[:, k, m * P:(m + 1) * P],
                            rhs=xtile[:, k, :],
                            start=(k == 0),
                            stop=(k == KO - 1),
                        )
                    nc.scalar.activation(
                        gitile[:, m, :], ps[:], AFT.Identity,
                        bias=bihe_sb[:, m:m + 1],
                    )

            def emit_step(gi_t, yw_t, tw, h_prev):
                c0 = tw * BL
                ghrz = ghpool.tile([P, 8, BL], f32, tag="ghrz")
                ghn = ghpool.tile([P, 4, BL], f32, tag="ghn")
                for m in range(8):
                    for k in range(KO):
                        nc.tensor.matmul(
                            ghrz[:, m, :],
                            lhsT=whh_sb[:, k, m * P:(m + 1) * P],
                            rhs=h_prev[:, k, :],
                            start=(k == 0),
                            stop=(k == KO - 1),
                        )
                for m in range(8, GM):
                    for k in range(KO):
                        nc.tensor.matmul(
                            ghn[:, m - 8, :],
                            lhsT=whh_sb[:, k, m * P:(m + 1) * P],
                            rhs=h_prev[:, k, :],
                            start=(k == 0),
                            stop=(k == KO - 1),
                        )
                rz = steppool.tile([P, 8, BL], f32, tag="rz")
                nsb = steppool.tile([P, 4, BL], f32, tag="nsb")
                tmp = steppool.tile([P, 4, BL], f32, tag="tmp")
                nc.vector.tensor_add(ghrz[:], ghrz[:], gi_t[:, 0:8, c0:c0 + BL])
                nc.scalar.activation(rz[:], ghrz[:], AFT.Sigmoid)
                nc.vector.tensor_add(ghn[:], ghn[:], bhhn_sb[:])
                nc.vector.tensor_mul(ghn[:], rz[:, 0:4, :], ghn[:])
                nc.vector.tensor_add(ghn[:], ghn[:], gi_t[:, 8:GM, c0:c0 + BL])
                nc.scalar.activation(nsb[:], ghn[:], AFT.Tanh)
                h_new = yw_t[:, :, c0:c0 + BL]
                nc.vector.tensor_sub(tmp[:], h_prev[:], nsb[:])
                nc.vector.tensor_mul(tmp[:], rz[:, 4:8, :], tmp[:])
                nc.vector.tensor_add(h_new, tmp[:], nsb[:])
                return h_new

            def emit_window(gi_cur, yw_cur, h_prev, xt_next, gi_next):
                for tw in range(8):
                    h_prev = emit_step(gi_cur, yw_cur, tw, h_prev)
                emit_gi(xt_next, gi_next)
                for tw in range(8, W):
                    h_prev = emit_step(gi_cur, yw_cur, tw, h_prev)
                return h_prev

            # prologue: first window's inputs + seed h into yw_b[1] tail
            nc.sync.dma_start(xt_b[0][:], xt_r[:, :, 0:WC])
            emit_gi(xt_b[0], gi_b[0])
            hseed = yw_b[1][:, :, (W - 1) * BL:]
            nc.vector.tensor_copy(hseed, h0_sb[:])
            h_prev = hseed

            def emit_body(col_a, h_prev):
                nc.sync.dma_start(xt_b[1][:], xt_r[:, :, ds(col_a + WC, WC)])
                h_prev = emit_window(gi_b[0], yw_b[0], h_prev, xt_b[1], gi_b[1])
                nc.sync.dma_start(yt_r[:, :, ds(col_a, WC)], yw_b[0][:])
                nc.sync.dma_start(xt_b[0][:], xt_r[:, :, ds(col_a + 2 * WC, WC)])
                h_prev = emit_window(gi_b[1], yw_b[1], h_prev, xt_b[0], gi_b[0])
                nc.sync.dma_start(yt_r[:, :, ds(col_a + WC, WC)], yw_b[1][:])
                return h_prev

            if loop_mode:
                import concourse.mybir as _m
                with tc.For_i(
                    0, NW // 2, 1,
                    hint_engines=(
                        _m.EngineType.PE,
                        _m.EngineType.DVE,
                        _m.EngineType.Activation,
                    ),
                ) as i:
                    emit_body(i * (2 * WC), h_prev)
            else:
                for i in range(NW // 2):
                    h_prev = emit_body(i * (2 * WC), h_prev)

            nc.vector.tensor_copy(hout_sb[:], hseed)
            nc.sync.dma_start(hTo[:].rearrange("p (k b) -> p k b", b=BL), hout_sb[:])

    nc.compile()
    return nc


def _get_module(T_, loop_mode, gi_f32r=False):
    key = (T_, loop_mode, gi_f32r)
    if key not in _BUILT:
        _BUILT[key] = _build(T_, loop_mode, gi_f32r)
    return _BUILT[key]


def _prep_core(x_dir, h0_dir, wih, bih, whh, bhh, bsl, T_):
    """Build the per-core input map. x_dir: [T_, B, F] already time-oriented."""
    xs = np.ascontiguousarray(x_dir[:, bsl, :])          # [T_, 8, F]
    xTl = np.ascontiguousarray(xs.reshape(T_ * BL, F).T)  # [F, T_*8]
    xTl = np.concatenate(
        [xTl, np.zeros((F, W * BL), np.float32)], axis=1)  # pad one window
    be = bih.copy()
    be[:2 * H] += bhh[:2 * H]
    bihe = np.ascontiguousarray(be.reshape(GM, P).T)     # [128, 12]
    bhhn = np.ascontiguousarray(
        np.broadcast_to(bhh[2 * H:].reshape(4, P).T[:, :, None], (P, 4, BL))
    ).reshape(P, 4 * BL)
    hs = h0_dir[bsl]                                     # [8, H]
    h0t = np.ascontiguousarray(
        hs.T.reshape(KO, P, BL).transpose(1, 0, 2)).reshape(P, KO * BL)
    return {
        "xT": xTl.astype(np.float32),
        "wihT": np.ascontiguousarray(wih.T).astype(np.float32),
        "whhT": np.ascontiguousarray(whh.T).astype(np.float32),
        "bihe": bihe.astype(np.float32),
        "bhhn": bhhn.astype(np.float32),
        "h0t": h0t.astype(np.float32),
    }


LAST_EXEC_NS = None
LAST_RESULTS = None


def _run(inputs, T_, loop_mode=True, trace=False, gi_f32r=False):
    global LAST_EXEC_NS, LAST_RESULTS
    from concourse import bass_utils

    nc = _get_module(T_, loop_mode, gi_f32r)

    x = np.asarray(inputs["inputs"], np.float32)[:T_]
    h0 = np.asarray(inputs["h0"], np.float32)
    xf = x
    xr = x[::-1]
    in_maps = []
    for c in range(8):
        fwd = c < 4
        q = c % 4
        bsl = slice(q * BL, (q + 1) * BL)
        if fwd:
            in_maps.append(_prep_core(
                xf, h0[0, 0], np.asarray(inputs["w_ih_f"]),
                np.asarray(inputs["b_ih_f"]), np.asarray(inputs["w_hh_f"]),
                np.asarray(inputs["b_hh_f"]), bsl, T_))
        else:
            in_maps.append(_prep_core(
                xr, h0[0, 1], np.asarray(inputs["w_ih_r"]),
                np.asarray(inputs["b_ih_r"]), np.asarray(inputs["w_hh_r"]),
                np.asarray(inputs["b_hh_r"]), bsl, T_))

    br = bass_utils.run_bass_kernel_spmd(
        nc, in_maps, core_ids=list(range(8)), trace=trace)
    LAST_EXEC_NS = br.exec_time_ns
    LAST_RESULTS = br

    output = np.empty((T_, B, 2 * H), np.float32)
    states = np.empty((1, 2, B, H), np.float32)
    for c in range(8):
        fwd = c < 4
        q = c % 4
        bsl = slice(q * BL, (q + 1) * BL)
        r = br.results[c]
        y = r["yT"].reshape(H, T_, BL).transpose(1, 2, 0)  # [T,8,H]
        hf = r["hTo"].reshape(P, KO, BL).transpose(2, 1, 0).reshape(BL, H)
        if fwd:
            output[:, bsl, :H] = y
            states[0, 0, bsl] = hf
        else:
            output[:, bsl, H:] = y
            states[0, 1, bsl] = hf
    return output, states


def kernel(**inputs):
    return _run(inputs, T, loop_mode=True)


# revision 4
# speedup vs baseline: 4.7179x; 4.7179x over previous
"""Bidirectional GRU (T=2048, B=32, F=H=512) on 8 Trainium2 NeuronCores.

Sharding: 8 cores = 2 directions x 4 batch quarters (B_local=8 per core).
Each core runs the full serial T-step recurrence for its slice; the reverse
direction cores receive time-flipped inputs so the same SPMD program serves
both directions.

Per-core layout: gates live as [128-gate-chunk partitions, batch] tiles so
all elementwise work runs at 128-partition occupancy. The input projection
gi = x @ w_ih^T + b is produced on-core in 64-step windows, double-buffered,
overlapped with the recurrence. Recurrent matmul: out[g,b] += w_hhT[k,g]^T
h[k,b] -> 48 small PE matmuls per step (weights stationary).
"""

import sys

import numpy as np

sys.path.insert(0, "/opt/trn_rl_repo")

T, B, F, H = 2048, 32, 512, 512
G = 3 * H
P = 128
BL = 8          # per-core batch
KO = F // P     # 4 contraction chunks
GM = G // P     # 12 gate chunks
W = 64          # steps per window (2 windows per loop body)

_BUILT = {}


def _build(T_, loop_mode, gi_f32r=False):
    import time as _time
    _t0 = _time.time()
    import concourse.mybir as mybir
    import concourse.tile as tile
    from concourse import bacc
    from concourse.bass import ds

    f32 = mybir.dt.float32
    gidt = mybir.dt.float32r if gi_f32r else mybir.dt.float32
    AFT = mybir.ActivationFunctionType
    NW = T_ // W
    assert NW % 2 == 0
    WC = W * BL  # columns per window (512)

    nc = bacc.Bacc("TRN2", target_bir_lowering=False, debug=False)

    xT = nc.dram_tensor("xT", [F, T_ * BL + WC], gidt, kind="ExternalInput")
    wihT = nc.dram_tensor("wihT", [F, G], gidt, kind="ExternalInput")
    whhT = nc.dram_tensor("whhT", [F, G], f32, kind="ExternalInput")
    bihe = nc.dram_tensor("bihe", [P, GM], f32, kind="ExternalInput")
    bhhn = nc.dram_tensor("bhhn", [P, 4 * BL], f32, kind="ExternalInput")
    h0t = nc.dram_tensor("h0t", [P, KO * BL], f32, kind="ExternalInput")
    yT = nc.dram_tensor("yT", [H, T_ * BL], f32, kind="ExternalOutput")
    hTo = nc.dram_tensor("hTo", [P, KO * BL], f32, kind="ExternalOutput")

    xt_r = xT[:].rearrange("(ko p) n -> p ko n", p=P)
    yt_r = yT[:].rearrange("(ko p) n -> p ko n", p=P)

    with tile.TileContext(nc) as tc:
        with (
            tc.tile_pool(name="const", bufs=1) as const,
            tc.tile_pool(name="dbuf", bufs=1) as dbuf,
            tc.tile_pool(name="step", bufs=3) as steppool,
            tc.tile_pool(name="ghps", bufs=2, space="PSUM") as ghpool,
            tc.tile_pool(name="gips", bufs=2, space="PSUM") as gipool,
        ):
            wih_sb = const.tile([P, KO, G], gidt, tag="wih", name="wih_sb")
            whh_sb = const.tile([P, KO, G], f32, tag="whh", name="whh_sb")
            bihe_sb = const.tile([P, GM], f32, tag="bihe", name="bihe_sb")
            bhhn_sb = const.tile([P, 4, BL], f32, tag="bhhn", name="bhhn_sb")
            h0_sb = const.tile([P, KO, BL], f32, tag="h0", name="h0_sb")
            hout_sb = const.tile([P, KO, BL], f32, tag="hout", name="hout_sb")

            xt_b = [dbuf.tile([P, KO, WC], gidt, tag=f"xt{i}", name=f"xt{i}") for i in range(2)]
            gi_b = [dbuf.tile([P, GM, WC], f32, tag=f"gi{i}", name=f"gi{i}") for i in range(2)]
            yw_b = [dbuf.tile([P, KO, WC], f32, tag=f"yw{i}", name=f"yw{i}") for i in range(2)]

            nc.sync.dma_start(wih_sb[:], wihT[:].rearrange("(ko p) g -> p ko g", p=P))
            nc.sync.dma_start(whh_sb[:], whhT[:].rearrange("(ko p) g -> p ko g", p=P))
            nc.sync.dma_start(bihe_sb[:], bihe[:])
            nc.sync.dma_start(bhhn_sb[:], bhhn[:].rearrange("p (k b) -> p k b", b=BL))
            nc.sync.dma_start(h0_sb[:], h0t[:].rearrange("p (k b) -> p k b", b=BL))

            def emit_gi(xtile, gitile):
                # gi^T window: [12 gate chunks x 128p, 512 cols] = wihT^T @ xT
                for m in range(GM):
                    ps = gipool.tile([P, WC], f32, tag="gips", name="gips")
                    for k in range(KO):
                        nc.tensor.matmul(
                            ps[:],
                            lhsT=wih_sb[:, k, m * P:(m + 1) * P],
                            rhs=xtile[:, k, :],
                            start=(k == 0),
                            stop=(k == KO - 1),
                        )
                    nc.scalar.activation(
                        gitile[:, m, :], ps[:], AFT.Identity,
                        bias=bihe_sb[:, m:m + 1],
                    )

            def emit_step(gi_t, yw_t, tw, h_prev):
                c0 = tw * BL
                ghrz = ghpool.tile([P, 8, BL], f32, tag="ghrz", name="ghrz")
                ghn = ghpool.tile([P, 4, BL], f32, tag="ghn", name="ghn")
                for m in range(8):
                    for k in range(KO):
                        nc.tensor.matmul(
                            ghrz[:, m, :],
                            lhsT=whh_sb[:, k, m * P:(m + 1) * P],
                            rhs=h_prev[:, k, :],
                            start=(k == 0),
                            stop=(k == KO - 1),
                        )
                for m in range(8, GM):
                    for k in range(KO):
                        nc.tensor.matmul(
                            ghn[:, m - 8, :],
                            lhsT=whh_sb[:, k, m * P:(m + 1) * P],
                            rhs=h_prev[:, k, :],
                            start=(k == 0),
                            stop=(k == KO - 1),
                        )
                rz = steppool.tile([P, 8, BL], f32, tag="rz", name="rz")
                nsb = steppool.tile([P, 4, BL], f32, tag="nsb", name="nsb")
                tmp = steppool.tile([P, 4, BL], f32, tag="tmp", name="tmp")
                nc.vector.tensor_add(ghrz[:], ghrz[:], gi_t[:, 0:8, c0:c0 + BL])
                nc.scalar.activation(rz[:], ghrz[:], AFT.Sigmoid)
                nc.vector.tensor_add(ghn[:], ghn[:], bhhn_sb[:])
                nc.vector.tensor_mul(ghn[:], rz[:, 0:4, :], ghn[:])
                nc.vector.tensor_add(ghn[:], ghn[:], gi_t[:, 8:GM, c0:c0 + BL])
                nc.scalar.activation(nsb[:], ghn[:], AFT.Tanh)
                h_new = yw_t[:, :, c0:c0 + BL]
                nc.vector.tensor_sub(tmp[:], h_prev[:], nsb[:])
                nc.vector.tensor_mul(tmp[:], rz[:, 4:8, :], tmp[:])
                nc.vector.tensor_add(h_new, tmp[:], nsb[:])
                return h_new

            def emit_window(gi_cur, yw_cur, h_prev, xt_next, gi_next):
                for tw in range(8):
                    h_prev = emit_step(gi_cur, yw_cur, tw, h_prev)
                emit_gi(xt_next, gi_next)
                for tw in range(8, W):
                    h_prev = emit_step(gi_cur, yw_cur, tw, h_prev)
                return h_prev

            # prologue: first window's inputs + seed h into yw_b[1] tail
            nc.sync.dma_start(xt_b[0][:], xt_r[:, :, 0:WC])
            emit_gi(xt_b[0], gi_b[0])
            hseed = yw_b[1][:, :, (W - 1) * BL:]
            nc.vector.tensor_copy(hseed, h0_sb[:])
            h_prev = hseed

            def emit_body(col_a, h_prev):
                nc.sync.dma_start(xt_b[1][:], xt_r[:, :, ds(col_a + WC, WC)])
                h_prev = emit_window(gi_b[0], yw_b[0], h_prev, xt_b[1], gi_b[1])
                nc.sync.dma_start(yt_r[:, :, ds(col_a, WC)], yw_b[0][:])
                nc.sync.dma_start(xt_b[0][:], xt_r[:, :, ds(col_a + 2 * WC, WC)])
                h_prev = emit_window(gi_b[1], yw_b[1], h_prev, xt_b[0], gi_b[0])
                nc.sync.dma_start(yt_r[:, :, ds(col_a + WC, WC)], yw_b[1][:])
                return h_prev

            if loop_mode:
                import concourse.mybir as _m
                with tc.For_i(
                    0, NW // 2, 1,
                    hint_engines=(
                        _m.EngineType.PE,
                        _m.EngineType.DVE,
                        _m.EngineType.Activation,
                    ),
                ) as i:
                    emit_body(i * (2 * WC), h_prev)
            else:
                for i in range(NW // 2):
                    h_prev = emit_body(i * (2 * WC), h_prev)

            nc.vector.tensor_copy(hout_sb[:], hseed)
            nc.sync.dma_start(hTo[:].rearrange("p (k b) -> p k b", b=BL), hout_sb[:])

    print(f"[build] emitted in {_time.time()-_t0:.1f}s", flush=True)
    nc.compile()
    print(f"[build] bacc-compiled in {_time.time()-_t0:.1f}s", flush=True)
    return nc


def _get_module(T_, loop_mode, gi_f32r=False):
    key = (T_, loop_mode, gi_f32r)
    if key not in _BUILT:
        _BUILT[key] = _build(T_, loop_mode, gi_f32r)
    return _BUILT[key]


def _prep_core(x_dir, h0_dir, wih, bih, whh, bhh, bsl, T_):
    """Build the per-core input map. x_dir: [T_, B, F] already time-oriented."""
    xs = np.ascontiguousarray(x_dir[:, bsl, :])          # [T_, 8, F]
    xTl = np.ascontiguousarray(xs.reshape(T_ * BL, F).T)  # [F, T_*8]
    xTl = np.concatenate(
        [xTl, np.zeros((F, W * BL), np.float32)], axis=1)  # pad one window
    be = bih.copy()
    be[:2 * H] += bhh[:2 * H]
    bihe = np.ascontiguousarray(be.reshape(GM, P).T)     # [128, 12]
    bhhn = np.ascontiguousarray(
        np.broadcast_to(bhh[2 * H:].reshape(4, P).T[:, :, None], (P, 4, BL))
    ).reshape(P, 4 * BL)
    hs = h0_dir[bsl]                                     # [8, H]
    h0t = np.ascontiguousarray(
        hs.T.reshape(KO, P, BL).transpose(1, 0, 2)).reshape(P, KO * BL)
    return {
        "xT": xTl.astype(np.float32),
        "wihT": np.ascontiguousarray(wih.T).astype(np.float32),
        "whhT": np.ascontiguousarray(whh.T).astype(np.float32),
        "bihe": bihe.astype(np.float32),
        "bhhn": bhhn.astype(np.float32),
        "h0t": h0t.astype(np.float32),
    }


LAST_EXEC_NS = None
LAST_RESULTS = None


def _make_in_maps(inputs, T_):
    x = np.asarray(inputs["inputs"], np.float32)[:T_]
    h0 = np.asarray(inputs["h0"], np.float32)
    xf = x
    xr = x[::-1]
    in_maps = []
    for c in range(8):
        fwd = c < 4
        q = c % 4
        bsl = slice(q * BL, (q + 1) * BL)
        if fwd:
            in_maps.append(_prep_core(
                xf, h0[0, 0], np.asarray(inputs["w_ih_f"]),
                np.asarray(inputs["b_ih_f"]), np.asarray(inputs["w_hh_f"]),
                np.asarray(inputs["b_hh_f"]), bsl, T_))
        else:
            in_maps.append(_prep_core(
                xr, h0[0, 1], np.asarray(inputs["w_ih_r"]),
                np.asarray(inputs["b_ih_r"]), np.asarray(inputs["w_hh_r"]),
                np.asarray(inputs["b_hh_r"]), bsl, T_))
    return in_maps


def _run(inputs, T_, loop_mode=True, trace=False, gi_f32r=False):
    global LAST_EXEC_NS, LAST_RESULTS
    from concourse import bass_utils

    nc = _get_module(T_, loop_mode, gi_f32r)
    in_maps = _make_in_maps(inputs, T_)

    import time as _time
    _t1 = _time.time()
    print("[run] dispatching to hw", flush=True)
    br = bass_utils.run_bass_kernel_spmd(
        nc, in_maps, core_ids=list(range(8)), trace=trace)
    print(f"[run] hw call done in {_time.time()-_t1:.1f}s", flush=True)
    LAST_EXEC_NS = br.exec_time_ns
    LAST_RESULTS = br

    output = np.empty((T_, B, 2 * H), np.float32)
    states = np.empty((1, 2, B, H), np.float32)
    for c in range(8):
        fwd = c < 4
        q = c % 4
        bsl = slice(q * BL, (q + 1) * BL)
        r = br.results[c]
        y = r["yT"].reshape(H, T_, BL).transpose(1, 2, 0)  # [T,8,H]
        hf = r["hTo"].reshape(P, KO, BL).transpose(2, 1, 0).reshape(BL, H)
        if fwd:
            output[:, bsl, :H] = y
            states[0, 0, bsl] = hf
        else:
            output[:, bsl, H:] = y
            states[0, 1, bsl] = hf
    return output, states


def kernel(**inputs):
    return _run(inputs, T, loop_mode=True)
